# revision 24
# baseline (speedup 1.0000x reference)
"""Trainium2 Bass kernel for nn_AlignModalities (ragged nearest-neighbor
resize of conv1x1(tgt) + LayerNorm).

Sharding: pure data parallel — batch row b -> core b (B == 8 == n_cores).

Per-core device program (SPMD — identical program, per-core data):
  1. y[t, o] = sum_c tgt[t, c] * W[o, c] + b[o]   (fp16-operand matmuls on
     PE accumulating in fp32 PSUM, bias folded in as a K=1 matmul against a
     ones-row)
  2. LayerNorm over channels via bn_stats/bn_aggr + per-partition
     scale/bias on the scalar engine (z = (y - mu) * rstd)
  3. Ragged nearest-neighbor row replication straight out of SBUF with
     indirect scatter-DMA: copy c of z-row t lands at out row start[t] + c;
     entries with copy index >= repeat count carry an OOB sentinel and are
     silently skipped (bounds_check).  Out rows past src_size are never
     written (output buffers are pre-zeroed).  All scatters live in one
     tile_critical with manual semaphores: paced by the per-tile z
     semaphore, completion via a dedicated DMA semaphore — this avoids the
     per-call write-after-write completion stalls the scheduler would
     otherwise insert on the shared output tensor (3.6us -> 1.6us/call).

All ragged/data-dependent structure lives in input tensors (offset tables),
so one compiled NEFF serves all 8 cores.
"""

from contextlib import ExitStack

import numpy as np

import concourse.bacc as bacc
import concourse.bass as bass
import concourse.tile as tile
from concourse import mybir
from concourse.bass_utils import run_bass_kernel_spmd

# Problem shape (hardcoded per contract).
B = 8
T_SRC = 4096
T_TGT = 2048
C = 1024
LN_EPS = 1e-5
N_CORES = 8

MAX_REP = 4  # max nearest-neighbor repeat count: ceil(T_SRC / (T_TGT/2))
N_TT = T_TGT // 128  # 16 token tiles of 128 rows
OOB_SENTINEL = 1 << 20

F32 = mybir.dt.float32
F16 = mybir.dt.float16
I32 = mybir.dt.int32

MM_NP_DT = np.float16  # host-side dtype for matmul operands


def _build_nc(ln_affine: bool, ln_bias_pad: bool):
    """Emit the single-core program. ln_affine: apply ln_g/ln_b elementwise.
    ln_bias_pad: write ln_b into out rows >= src_size (only needed when
    ln_b != 0)."""
    nc = bacc.Bacc("TRN2", target_bir_lowering=False, debug=False,
                   num_devices=N_CORES)

    tgtT = nc.dram_tensor("tgtT", [C, T_TGT], F16, kind="ExternalInput")
    wT = nc.dram_tensor("wT", [C, C], F16, kind="ExternalInput")
    cb = nc.dram_tensor("cb", [1, C], F16, kind="ExternalInput")
    ones = nc.dram_tensor("ones", [1, 128], F16, kind="ExternalInput")
    offs = nc.dram_tensor("offs", [128, MAX_REP, N_TT], I32,
                          kind="ExternalInput")
    sflags = nc.dram_tensor("sflags", [1, 4], I32, kind="ExternalInput")
    if ln_affine:
        lng = nc.dram_tensor("lng", [1, C], F32, kind="ExternalInput")
        lnb = nc.dram_tensor("lnb", [1, C], F32, kind="ExternalInput")
    if ln_bias_pad:
        pad_offs = nc.dram_tensor("pad_offs", [128, N_TT], I32,
                                  kind="ExternalInput")
    out_t = nc.dram_tensor("out", [T_SRC, C], F32, kind="ExternalOutput")
    out_full = out_t.ap()

    dsem = nc.alloc_semaphore("dsem")

    with tile.TileContext(nc) as tc, ExitStack() as ctx:
        const = ctx.enter_context(tc.tile_pool(name="const", bufs=1))
        zpool = ctx.enter_context(tc.tile_pool(name="z", bufs=N_TT))
        psum = ctx.enter_context(tc.tile_pool(name="psum", bufs=6,
                                              space="PSUM"))
        stats = ctx.enter_context(tc.tile_pool(name="stats", bufs=4))

        # ---- resident inputs (small constants first: the bias matmuls
        # and scatters need them; then tgtT/wT split fine so the first
        # matmuls start early) ----
        cb_sb = const.tile([1, C], F16)
        nc.sync.dma_start(cb_sb[:], cb.ap())
        offs_sb = const.tile([128, MAX_REP, N_TT], I32)
        nc.sync.dma_start(offs_sb[:], offs.ap())
        sflags_sb = const.tile([1, 4], I32)
        nc.sync.dma_start(sflags_sb[:], sflags.ap())
        ones_sb = const.tile([1, 128], F16)
        nc.sync.dma_start(ones_sb[:], ones.ap())
        eps_sb = const.tile([128, 1], F32)
        nc.vector.memset(eps_sb[:], LN_EPS)
        tgtT_sb = const.tile([128, 8, T_TGT], F16)
        tgtT_r = tgtT.ap().rearrange("(m p) t -> p m t", p=128)
        wT_sb = const.tile([128, 8, C], F16)
        wT_r = wT.ap().rearrange("(m p) o -> p m o", p=128)
        QT = T_TGT // 8
        # first matmul needs only wT chunk 0 + tgtT tile 0: land those first
        nc.sync.dma_start(wT_sb[:, 0, :], wT_r[:, 0, :])
        nc.sync.dma_start(tgtT_sb[:, :, 0:128], tgtT_r[:, :, 0:128])
        for m in range(1, 8):
            nc.sync.dma_start(wT_sb[:, m, :], wT_r[:, m, :])
        nc.sync.dma_start(tgtT_sb[:, :, 128:QT], tgtT_r[:, :, 128:QT])
        for q in range(1, 8):
            nc.sync.dma_start(tgtT_sb[:, :, q * QT:(q + 1) * QT],
                              tgtT_r[:, :, q * QT:(q + 1) * QT])

        pass

        if ln_affine:
            g_sb = const.tile([128, C], F32)
            nc.sync.dma_start(g_sb[:], lng.ap().to_broadcast([128, C]))
            b_sb = const.tile([128, C], F32)
            nc.sync.dma_start(b_sb[:], lnb.ap().to_broadcast([128, C]))
        if ln_bias_pad:
            pad_offs_sb = const.tile([128, N_TT], I32)
            nc.sync.dma_start(pad_offs_sb[:], pad_offs.ap())

        # ---- per token-tile compute ----
        z_tiles = []
        for ti in range(N_TT):
            t_lo = ti * 128
            ps = [psum.tile([128, 512], F32, tag="ps", name=f"ps{ti}_{o}")
                  for o in range(2)]
            for o in range(2):
                for m in range(8):
                    nc.tensor.matmul(
                        ps[o][:],
                        lhsT=tgtT_sb[:, m, t_lo:t_lo + 128],
                        rhs=wT_sb[:, m, o * 512:(o + 1) * 512],
                        start=(m == 0), stop=False)
                # bias via K=1 matmul: ones[1,128].T @ conv_b[1,512]
                nc.tensor.matmul(
                    ps[o][:],
                    lhsT=ones_sb[:, :],
                    rhs=cb_sb[:, o * 512:(o + 1) * 512],
                    start=False, stop=True)

            st = stats.tile([128, 2, 6], F32, tag="st")
            nc.vector.bn_stats(st[:, 0, :], ps[0][:])
            nc.vector.bn_stats(st[:, 1, :], ps[1][:])
            mv = stats.tile([128, 2], F32, tag="mv")
            nc.vector.bn_aggr(mv[:], st[:])
            kt = stats.tile([128, 1], F32, tag="kt")
            nc.scalar.activation(kt[:], mv[:, 1:2],
                                 mybir.ActivationFunctionType.Sqrt,
                                 bias=eps_sb[:], scale=1.0)
            nc.vector.reciprocal(kt[:], kt[:])
            nmk = stats.tile([128, 1], F32, tag="nmk")
            nc.vector.tensor_tensor(nmk[:], mv[:, 0:1], kt[:],
                                    op=mybir.AluOpType.mult)
            nc.scalar.mul(nmk[:], nmk[:], -1.0)

            z = zpool.tile([128, C], F32, tag="z", name=f"z{ti}")
            for o in range(2):
                nc.scalar.activation(z[:, o * 512:(o + 1) * 512], ps[o][:],
                                     mybir.ActivationFunctionType.Identity,
                                     bias=nmk[:], scale=kt[:])
            if ln_affine:
                nc.vector.tensor_tensor(z[:], z[:], g_sb[:],
                                        op=mybir.AluOpType.mult)
                nc.vector.tensor_tensor(z[:], z[:], b_sb[:],
                                        op=mybir.AluOpType.add)
            z_tiles.append(z)

        # ---- replicated scatter stream ----
        # copy c of z row t -> out row start[t] + c; OOB entries skipped.
        # Grouped into criticals of GRP token tiles: each group's Tile-level
        # dependency is only its own z tiles, so scatters overlap the rest
        # of the compute; inside a critical the scheduler adds no per-call
        # completion waits (the 2.6x serialization fix).
        GRPS = [(g, 2) for g in range(0, N_TT, 2)]

        def scat(ti, c):
            nc.gpsimd.indirect_dma_start(
                out=out_full,
                out_offset=bass.IndirectOffsetOnAxis(
                    ap=offs_sb[:, c, ti:ti + 1], axis=0),
                in_=z_tiles[ti][:],
                in_offset=None,
                bounds_check=T_SRC - 1,
                oob_is_err=False).then_inc(dsem, 16)

        # registers: active group count, max repeat, dsem completion target
        with nc.gpsimd.register("rg") as rg, \
             nc.gpsimd.register("rc") as rc, \
             nc.gpsimd.register("rt") as rt:
            for g, (g0, gsz) in enumerate(GRPS):
                with tc.tile_critical(no_gpsimd_drain=True):
                    if g == 0:
                        nc.gpsimd.reg_load(rg, sflags_sb[0:1, 0:1])
                        nc.gpsimd.reg_load(rc, sflags_sb[0:1, 1:2])
                        nc.gpsimd.reg_load(rt, sflags_sb[0:1, 2:3])
                    with nc.gpsimd.If_cmp(rg, g, "IS_GT"):
                        for ti in range(g0, g0 + gsz):
                            scat(ti, 0)
                            scat(ti, 1)
                        with nc.gpsimd.If_cmp(rc, 2, "IS_GT"):
                            for ti in range(g0, g0 + gsz):
                                scat(ti, 2)
                            with nc.gpsimd.If_cmp(rc, 3, "IS_GT"):
                                for ti in range(g0, g0 + gsz):
                                    scat(ti, 3)
            with tc.tile_critical(no_gpsimd_drain=True):  # dsem wait below
                # already proves every scatter DMA completed
                if ln_bias_pad:
                    for e in range(N_TT):
                        nc.gpsimd.indirect_dma_start(
                            out=out_full,
                            out_offset=bass.IndirectOffsetOnAxis(
                                ap=pad_offs_sb[:, e:e + 1], axis=0),
                            in_=b_sb[:],
                            in_offset=None,
                            bounds_check=T_SRC - 1,
                            oob_is_err=False).then_inc(dsem, 16)
                nc.gpsimd.wait_ge(dsem, rt)

    nc.compile()
    return nc


_PLANS: dict = {}


def _get_plan(ln_affine: bool, ln_bias_pad: bool):
    key = (ln_affine, ln_bias_pad)
    if key not in _PLANS:
        _PLANS[key] = _build_nc(ln_affine, ln_bias_pad)
    return _PLANS[key]


def _host_prep(src_mask, tgt_mask):
    """Per-row scatter offset tables from the ragged masks."""
    src_sizes = src_mask.astype(np.int64).sum(-1)
    tgt_sizes = tgt_mask.astype(np.int64).sum(-1)
    offs = np.full((B, 128, MAX_REP, N_TT), OOB_SENTINEL, dtype=np.int32)
    pad_offs = np.full((B, 128, N_TT), OOB_SENTINEL, dtype=np.int32)
    flags = np.zeros((B, 3), dtype=np.int64)  # [n_active, max_rep, tiles]
    j = np.arange(T_SRC, dtype=np.int64)
    for b in range(B):
        ss, ts = int(src_sizes[b]), int(tgt_sizes[b])
        idx = (j * ts) // max(ss, 1)
        idx = np.clip(idx, 0, max(ts, 1) - 1)
        idxv = idx[:ss]
        counts = np.bincount(idxv, minlength=T_TGT)  # repeats per z row
        starts = np.concatenate(([0], np.cumsum(counts)[:-1]))
        tab = np.full((T_TGT, MAX_REP), OOB_SENTINEL, dtype=np.int64)
        for c in range(MAX_REP):
            m = counts > c
            tab[m, c] = starts[m] + c
        # t = 128*k + p  ->  [p, c, k]
        offs[b] = tab.reshape(N_TT, 128, MAX_REP).transpose(1, 2, 0)
        t_used = int(np.nonzero(counts)[0].max()) + 1 if counts.any() else 0
        tiles_used = -(-t_used // 128)
        grp_starts = list(range(0, N_TT, 2))
        grp_sizes = [2] * (N_TT // 2)
        n_active = sum(1 for s0 in grp_starts if s0 < tiles_used)
        flags[b, 0] = n_active                  # active scatter groups
        flags[b, 1] = int(counts.max())         # max repeat count
        flags[b, 2] = sum(sz for s0, sz in zip(grp_starts, grp_sizes)
                          if s0 < tiles_used)   # tiles emitted
        npad = T_SRC - ss
        padtab = np.full(128 * N_TT, OOB_SENTINEL, dtype=np.int64)
        padtab[:npad] = ss + np.arange(npad)
        pad_offs[b] = padtab.reshape(N_TT, 128).transpose(1, 0)
    return offs, pad_offs, flags


def _prepare(src, src_mask, tgt, tgt_mask, conv_w, conv_b, ln_g, ln_b):
    """Host-side sharding/layout prep. Returns (nc, in_maps, src_mask)."""
    src_mask = np.asarray(src_mask).astype(np.int32)
    tgt_mask = np.asarray(tgt_mask).astype(np.int32)
    tgt = np.asarray(tgt, dtype=np.float32)
    conv_w = np.asarray(conv_w, dtype=np.float32)
    conv_b = np.asarray(conv_b, dtype=np.float32)
    ln_g = np.asarray(ln_g, dtype=np.float32)
    ln_b = np.asarray(ln_b, dtype=np.float32)

    ln_affine = not (np.all(ln_g == 1.0) and np.all(ln_b == 0.0))
    ln_bias_pad = not np.all(ln_b == 0.0)
    nc = _get_plan(ln_affine, ln_bias_pad)

    offs, pad_offs, flags = _host_prep(src_mask, tgt_mask)
    wT = np.ascontiguousarray(conv_w.T).astype(MM_NP_DT)  # [C_in, C_out]
    cb = conv_b.reshape(1, C).astype(MM_NP_DT)
    ones = np.ones((1, 128), dtype=MM_NP_DT)

    sflags = np.zeros((B, 1, 4), dtype=np.int32)
    for b in range(B):
        ng, mr, nt = int(flags[b, 0]), int(flags[b, 1]), int(flags[b, 2])
        per_tile = 2 + (1 if mr >= 3 else 0) + (1 if mr >= 4 else 0)
        n_calls = nt * per_tile + (N_TT if ln_bias_pad else 0)
        sflags[b] = np.array([[ng, mr, 16 * n_calls, 0]], dtype=np.int32)

    in_maps = []
    for b in range(B):
        m = {
            "tgtT": np.ascontiguousarray(tgt[b].T).astype(MM_NP_DT),
            "wT": wT,
            "cb": cb,
            "offs": offs[b],
            "ones": ones,
            "sflags": sflags[b],
        }
        if ln_affine:
            m["lng"] = ln_g.reshape(1, C)
            m["lnb"] = ln_b.reshape(1, C)
        if ln_bias_pad:
            m["pad_offs"] = pad_offs[b]
        in_maps.append(m)
    return nc, in_maps, src_mask


def kernel(src, src_mask, tgt, tgt_mask, conv_w, conv_b, ln_g, ln_b):
    nc, in_maps, src_mask = _prepare(src, src_mask, tgt, tgt_mask,
                                     conv_w, conv_b, ln_g, ln_b)
    res = run_bass_kernel_spmd(nc, in_maps, list(range(N_CORES)))
    out = np.stack([res.results[i]["out"] for i in range(N_CORES)])
    return (out, src_mask)


# revision 25
# speedup vs baseline: 1.1169x; 1.1169x over previous
"""Trainium2 Bass kernel for nn_AlignModalities (ragged nearest-neighbor
resize of conv1x1(tgt) + LayerNorm).

Sharding: pure data parallel — batch row b -> core b (B == 8 == n_cores).

Per-core device program (SPMD — identical program, per-core data):
  1. y[t, o] = sum_c tgt[t, c] * W[o, c] + b[o]   (fp16-operand matmuls on
     PE accumulating in fp32 PSUM, bias folded in as a K=1 matmul against a
     ones-row)
  2. LayerNorm over channels via bn_stats/bn_aggr + per-partition
     scale/bias on the scalar engine (z = (y - mu) * rstd)
  3. Ragged nearest-neighbor row replication straight out of SBUF with
     indirect scatter-DMA: copy c of z-row t lands at out row start[t] + c;
     entries with copy index >= repeat count carry an OOB sentinel and are
     silently skipped (bounds_check).  Out rows past src_size are never
     written (output buffers are pre-zeroed).  All scatters live in one
     tile_critical with manual semaphores: paced by the per-tile z
     semaphore, completion via a dedicated DMA semaphore — this avoids the
     per-call write-after-write completion stalls the scheduler would
     otherwise insert on the shared output tensor (3.6us -> 1.6us/call).

All ragged/data-dependent structure lives in input tensors (offset tables),
so one compiled NEFF serves all 8 cores.
"""

from contextlib import ExitStack

import numpy as np

import concourse.bacc as bacc
import concourse.bass as bass
import concourse.tile as tile
from concourse import mybir
from concourse.bass_utils import run_bass_kernel_spmd

# Problem shape (hardcoded per contract).
B = 8
T_SRC = 4096
T_TGT = 2048
C = 1024
LN_EPS = 1e-5
N_CORES = 8

MAX_REP = 4  # max nearest-neighbor repeat count: ceil(T_SRC / (T_TGT/2))
N_TT = T_TGT // 128  # 16 token tiles of 128 rows
OOB_SENTINEL = 1 << 20

F32 = mybir.dt.float32
F16 = mybir.dt.float16
I32 = mybir.dt.int32

MM_NP_DT = np.float16  # host-side dtype for matmul operands


def _build_nc(ln_affine: bool, ln_bias_pad: bool):
    """Emit the single-core program. ln_affine: apply ln_g/ln_b elementwise.
    ln_bias_pad: write ln_b into out rows >= src_size (only needed when
    ln_b != 0)."""
    nc = bacc.Bacc("TRN2", target_bir_lowering=False, debug=False,
                   num_devices=N_CORES)

    tgtT = nc.dram_tensor("tgtT", [C, T_TGT], F16, kind="ExternalInput")
    wT = nc.dram_tensor("wT", [C, C], F16, kind="ExternalInput")
    cb = nc.dram_tensor("cb", [1, C], F16, kind="ExternalInput")
    ones = nc.dram_tensor("ones", [1, 128], F16, kind="ExternalInput")
    offs = nc.dram_tensor("offs", [128, MAX_REP, N_TT], I32,
                          kind="ExternalInput")
    sflags = nc.dram_tensor("sflags", [1, 4], I32, kind="ExternalInput")
    if ln_affine:
        lng = nc.dram_tensor("lng", [1, C], F32, kind="ExternalInput")
        lnb = nc.dram_tensor("lnb", [1, C], F32, kind="ExternalInput")
    if ln_bias_pad:
        pad_offs = nc.dram_tensor("pad_offs", [128, N_TT], I32,
                                  kind="ExternalInput")
    out_t = nc.dram_tensor("out", [T_SRC, C], F32, kind="ExternalOutput")
    out_full = out_t.ap()

    dsem = nc.alloc_semaphore("dsem")

    with tile.TileContext(nc) as tc, ExitStack() as ctx:
        const = ctx.enter_context(tc.tile_pool(name="const", bufs=1))
        zpool = ctx.enter_context(tc.tile_pool(name="z", bufs=N_TT))
        psum = ctx.enter_context(tc.tile_pool(name="psum", bufs=6,
                                              space="PSUM"))
        stats = ctx.enter_context(tc.tile_pool(name="stats", bufs=4))

        # ---- resident inputs (small constants first: the bias matmuls
        # and scatters need them; then tgtT/wT split fine so the first
        # matmuls start early) ----
        cb_sb = const.tile([1, C], F16)
        nc.sync.dma_start(cb_sb[:], cb.ap())
        offs_sb = const.tile([128, MAX_REP, N_TT], I32)
        nc.sync.dma_start(offs_sb[:], offs.ap())
        sflags_sb = const.tile([1, 4], I32)
        nc.sync.dma_start(sflags_sb[:], sflags.ap())
        ones_sb = const.tile([1, 128], F16)
        nc.sync.dma_start(ones_sb[:], ones.ap())
        eps_sb = const.tile([128, 1], F32)
        nc.vector.memset(eps_sb[:], LN_EPS)
        tgtT_sb = const.tile([128, 8, T_TGT], F16)
        tgtT_r = tgtT.ap().rearrange("(m p) t -> p m t", p=128)
        wT_sb = const.tile([128, 8, C], F16)
        wT_r = wT.ap().rearrange("(m p) o -> p m o", p=128)
        QT = T_TGT // 8
        # first matmul needs only wT chunk 0 + tgtT tile 0: land those first
        nc.sync.dma_start(wT_sb[:, 0, :], wT_r[:, 0, :])
        nc.sync.dma_start(tgtT_sb[:, :, 0:128], tgtT_r[:, :, 0:128])
        for m in range(1, 8):
            nc.sync.dma_start(wT_sb[:, m, :], wT_r[:, m, :])
        nc.sync.dma_start(tgtT_sb[:, :, 128:QT], tgtT_r[:, :, 128:QT])
        for q in range(1, 8):
            nc.sync.dma_start(tgtT_sb[:, :, q * QT:(q + 1) * QT],
                              tgtT_r[:, :, q * QT:(q + 1) * QT])

        pass

        if ln_affine:
            g_sb = const.tile([128, C], F32)
            nc.sync.dma_start(g_sb[:], lng.ap().to_broadcast([128, C]))
            b_sb = const.tile([128, C], F32)
            nc.sync.dma_start(b_sb[:], lnb.ap().to_broadcast([128, C]))
        if ln_bias_pad:
            pad_offs_sb = const.tile([128, N_TT], I32)
            nc.sync.dma_start(pad_offs_sb[:], pad_offs.ap())

        # ---- per token-tile compute ----
        z_tiles = []
        for ti in range(N_TT):
            t_lo = ti * 128
            ps = [psum.tile([128, 512], F32, tag="ps", name=f"ps{ti}_{o}")
                  for o in range(2)]
            for o in range(2):
                for m in range(8):
                    nc.tensor.matmul(
                        ps[o][:],
                        lhsT=tgtT_sb[:, m, t_lo:t_lo + 128],
                        rhs=wT_sb[:, m, o * 512:(o + 1) * 512],
                        start=(m == 0), stop=False)
                # bias via K=1 matmul: ones[1,128].T @ conv_b[1,512]
                nc.tensor.matmul(
                    ps[o][:],
                    lhsT=ones_sb[:, :],
                    rhs=cb_sb[:, o * 512:(o + 1) * 512],
                    start=False, stop=True)

            st = stats.tile([128, 2, 6], F32, tag="st")
            nc.vector.bn_stats(st[:, 0, :], ps[0][:])
            nc.vector.bn_stats(st[:, 1, :], ps[1][:])
            mv = stats.tile([128, 2], F32, tag="mv")
            nc.vector.bn_aggr(mv[:], st[:])
            kt = stats.tile([128, 1], F32, tag="kt")
            nc.scalar.activation(kt[:], mv[:, 1:2],
                                 mybir.ActivationFunctionType.Sqrt,
                                 bias=eps_sb[:], scale=1.0)
            nc.vector.reciprocal(kt[:], kt[:])
            nmk = stats.tile([128, 1], F32, tag="nmk")
            nc.vector.tensor_tensor(nmk[:], mv[:, 0:1], kt[:],
                                    op=mybir.AluOpType.mult)
            nc.scalar.mul(nmk[:], nmk[:], -1.0)

            z = zpool.tile([128, C], F32, tag="z", name=f"z{ti}")
            nc.scalar.activation(z[:, 0:512], ps[0][:],
                                 mybir.ActivationFunctionType.Identity,
                                 bias=nmk[:], scale=kt[:])
            nc.vector.tensor_scalar(out=z[:, 512:1024], in0=ps[1][:],
                                    scalar1=mv[:, 0:1], scalar2=kt[:],
                                    op0=mybir.AluOpType.subtract,
                                    op1=mybir.AluOpType.mult)
            if ln_affine:
                nc.vector.tensor_tensor(z[:], z[:], g_sb[:],
                                        op=mybir.AluOpType.mult)
                nc.vector.tensor_tensor(z[:], z[:], b_sb[:],
                                        op=mybir.AluOpType.add)
            z_tiles.append(z)

        # ---- replicated scatter stream ----
        # copy c of z row t -> out row start[t] + c; OOB entries skipped.
        # Grouped into criticals of GRP token tiles: each group's Tile-level
        # dependency is only its own z tiles, so scatters overlap the rest
        # of the compute; inside a critical the scheduler adds no per-call
        # completion waits (the 2.6x serialization fix).
        GRPS = [(g, 2) for g in range(0, N_TT, 2)]

        def scat(ti, c):
            nc.gpsimd.indirect_dma_start(
                out=out_full,
                out_offset=bass.IndirectOffsetOnAxis(
                    ap=offs_sb[:, c, ti:ti + 1], axis=0),
                in_=z_tiles[ti][:],
                in_offset=None,
                bounds_check=T_SRC - 1,
                oob_is_err=False).then_inc(dsem, 16)

        # registers: active group count, max repeat, dsem completion target
        with nc.gpsimd.register("rg") as rg, \
             nc.gpsimd.register("rc") as rc, \
             nc.gpsimd.register("rt") as rt:
            for g, (g0, gsz) in enumerate(GRPS):
                with tc.tile_critical(no_gpsimd_drain=True):
                    if g == 0:
                        nc.gpsimd.reg_load(rg, sflags_sb[0:1, 0:1])
                        nc.gpsimd.reg_load(rc, sflags_sb[0:1, 1:2])
                        nc.gpsimd.reg_load(rt, sflags_sb[0:1, 2:3])
                    with nc.gpsimd.If_cmp(rg, g, "IS_GT"):
                        for ti in range(g0, g0 + gsz):
                            scat(ti, 0)
                            scat(ti, 1)
                        with nc.gpsimd.If_cmp(rc, 2, "IS_GT"):
                            for ti in range(g0, g0 + gsz):
                                scat(ti, 2)
                            with nc.gpsimd.If_cmp(rc, 3, "IS_GT"):
                                for ti in range(g0, g0 + gsz):
                                    scat(ti, 3)
                    if g == len(GRPS) - 1:
                        # close open If bodies, then the unconditional
                        # pad scatters + completion wait (dsem proves all
                        # scatter DMAs landed; no extra critical needed)
                        nc.gpsimd.end_ifs()
                        if ln_bias_pad:
                            for e in range(N_TT):
                                nc.gpsimd.indirect_dma_start(
                                    out=out_full,
                                    out_offset=bass.IndirectOffsetOnAxis(
                                        ap=pad_offs_sb[:, e:e + 1], axis=0),
                                    in_=b_sb[:],
                                    in_offset=None,
                                    bounds_check=T_SRC - 1,
                                    oob_is_err=False).then_inc(dsem, 16)
                        nc.gpsimd.wait_ge(dsem, rt)

    nc.compile()
    return nc


_PLANS: dict = {}


def _get_plan(ln_affine: bool, ln_bias_pad: bool):
    key = (ln_affine, ln_bias_pad)
    if key not in _PLANS:
        _PLANS[key] = _build_nc(ln_affine, ln_bias_pad)
    return _PLANS[key]


def _host_prep(src_mask, tgt_mask):
    """Per-row scatter offset tables from the ragged masks."""
    src_sizes = src_mask.astype(np.int64).sum(-1)
    tgt_sizes = tgt_mask.astype(np.int64).sum(-1)
    offs = np.full((B, 128, MAX_REP, N_TT), OOB_SENTINEL, dtype=np.int32)
    pad_offs = np.full((B, 128, N_TT), OOB_SENTINEL, dtype=np.int32)
    flags = np.zeros((B, 3), dtype=np.int64)  # [n_active, max_rep, tiles]
    j = np.arange(T_SRC, dtype=np.int64)
    for b in range(B):
        ss, ts = int(src_sizes[b]), int(tgt_sizes[b])
        idx = (j * ts) // max(ss, 1)
        idx = np.clip(idx, 0, max(ts, 1) - 1)
        idxv = idx[:ss]
        counts = np.bincount(idxv, minlength=T_TGT)  # repeats per z row
        starts = np.concatenate(([0], np.cumsum(counts)[:-1]))
        tab = np.full((T_TGT, MAX_REP), OOB_SENTINEL, dtype=np.int64)
        for c in range(MAX_REP):
            m = counts > c
            tab[m, c] = starts[m] + c
        # t = 128*k + p  ->  [p, c, k]
        offs[b] = tab.reshape(N_TT, 128, MAX_REP).transpose(1, 2, 0)
        t_used = int(np.nonzero(counts)[0].max()) + 1 if counts.any() else 0
        tiles_used = -(-t_used // 128)
        grp_starts = list(range(0, N_TT, 2))
        grp_sizes = [2] * (N_TT // 2)
        n_active = sum(1 for s0 in grp_starts if s0 < tiles_used)
        flags[b, 0] = n_active                  # active scatter groups
        flags[b, 1] = int(counts.max())         # max repeat count
        flags[b, 2] = sum(sz for s0, sz in zip(grp_starts, grp_sizes)
                          if s0 < tiles_used)   # tiles emitted
        npad = T_SRC - ss
        padtab = np.full(128 * N_TT, OOB_SENTINEL, dtype=np.int64)
        padtab[:npad] = ss + np.arange(npad)
        pad_offs[b] = padtab.reshape(N_TT, 128).transpose(1, 0)
    return offs, pad_offs, flags


def _prepare(src, src_mask, tgt, tgt_mask, conv_w, conv_b, ln_g, ln_b):
    """Host-side sharding/layout prep. Returns (nc, in_maps, src_mask)."""
    src_mask = np.asarray(src_mask).astype(np.int32)
    tgt_mask = np.asarray(tgt_mask).astype(np.int32)
    tgt = np.asarray(tgt, dtype=np.float32)
    conv_w = np.asarray(conv_w, dtype=np.float32)
    conv_b = np.asarray(conv_b, dtype=np.float32)
    ln_g = np.asarray(ln_g, dtype=np.float32)
    ln_b = np.asarray(ln_b, dtype=np.float32)

    ln_affine = not (np.all(ln_g == 1.0) and np.all(ln_b == 0.0))
    ln_bias_pad = not np.all(ln_b == 0.0)
    nc = _get_plan(ln_affine, ln_bias_pad)

    offs, pad_offs, flags = _host_prep(src_mask, tgt_mask)
    wT = np.ascontiguousarray(conv_w.T).astype(MM_NP_DT)  # [C_in, C_out]
    cb = conv_b.reshape(1, C).astype(MM_NP_DT)
    ones = np.ones((1, 128), dtype=MM_NP_DT)

    sflags = np.zeros((B, 1, 4), dtype=np.int32)
    for b in range(B):
        ng, mr, nt = int(flags[b, 0]), int(flags[b, 1]), int(flags[b, 2])
        per_tile = 2 + (1 if mr >= 3 else 0) + (1 if mr >= 4 else 0)
        n_calls = nt * per_tile + (N_TT if ln_bias_pad else 0)
        sflags[b] = np.array([[ng, mr, 16 * n_calls, 0]], dtype=np.int32)

    in_maps = []
    for b in range(B):
        m = {
            "tgtT": np.ascontiguousarray(tgt[b].T).astype(MM_NP_DT),
            "wT": wT,
            "cb": cb,
            "offs": offs[b],
            "ones": ones,
            "sflags": sflags[b],
        }
        if ln_affine:
            m["lng"] = ln_g.reshape(1, C)
            m["lnb"] = ln_b.reshape(1, C)
        if ln_bias_pad:
            m["pad_offs"] = pad_offs[b]
        in_maps.append(m)
    return nc, in_maps, src_mask


def kernel(src, src_mask, tgt, tgt_mask, conv_w, conv_b, ln_g, ln_b):
    nc, in_maps, src_mask = _prepare(src, src_mask, tgt, tgt_mask,
                                     conv_w, conv_b, ln_g, ln_b)
    res = run_bass_kernel_spmd(nc, in_maps, list(range(N_CORES)))
    out = np.stack([res.results[i]["out"] for i in range(N_CORES)])
    return (out, src_mask)


# revision 26
# speedup vs baseline: 1.1659x; 1.0439x over previous
"""Trainium2 Bass kernel for nn_AlignModalities (ragged nearest-neighbor
resize of conv1x1(tgt) + LayerNorm).

Sharding: pure data parallel — batch row b -> core b (B == 8 == n_cores).

Per-core device program (SPMD — identical program, per-core data):
  1. y[t, o] = sum_c tgt[t, c] * W[o, c] + b[o]   (fp16-operand matmuls on
     PE accumulating in fp32 PSUM, bias folded in as a K=1 matmul against a
     ones-row)
  2. LayerNorm over channels via bn_stats/bn_aggr + per-partition
     scale/bias on the scalar engine (z = (y - mu) * rstd)
  3. Ragged nearest-neighbor row replication straight out of SBUF with
     indirect scatter-DMA: copy c of z-row t lands at out row start[t] + c;
     entries with copy index >= repeat count carry an OOB sentinel and are
     silently skipped (bounds_check).  Out rows past src_size are never
     written (output buffers are pre-zeroed).  All scatters live in one
     tile_critical with manual semaphores: paced by the per-tile z
     semaphore, completion via a dedicated DMA semaphore — this avoids the
     per-call write-after-write completion stalls the scheduler would
     otherwise insert on the shared output tensor (3.6us -> 1.6us/call).

All ragged/data-dependent structure lives in input tensors (offset tables),
so one compiled NEFF serves all 8 cores.
"""

from contextlib import ExitStack

import numpy as np

import concourse.bacc as bacc
import concourse.bass as bass
import concourse.tile as tile
from concourse import mybir
from concourse.bass_utils import run_bass_kernel_spmd

# Problem shape (hardcoded per contract).
B = 8
T_SRC = 4096
T_TGT = 2048
C = 1024
LN_EPS = 1e-5
N_CORES = 8

MAX_REP = 4  # max nearest-neighbor repeat count: ceil(T_SRC / (T_TGT/2))
N_TT = T_TGT // 128  # 16 token tiles of 128 rows
OOB_SENTINEL = 1 << 20

F32 = mybir.dt.float32
F16 = mybir.dt.float16
I32 = mybir.dt.int32

MM_NP_DT = np.float16  # host-side dtype for matmul operands


def _build_nc(ln_affine: bool, ln_bias_pad: bool):
    """Emit the single-core program. ln_affine: apply ln_g/ln_b elementwise.
    ln_bias_pad: write ln_b into out rows >= src_size (only needed when
    ln_b != 0)."""
    nc = bacc.Bacc("TRN2", target_bir_lowering=False, debug=False,
                   num_devices=N_CORES)

    tgtT = nc.dram_tensor("tgtT", [C, T_TGT], F16, kind="ExternalInput")
    wT = nc.dram_tensor("wT", [C, C], F16, kind="ExternalInput")
    cb = nc.dram_tensor("cb", [1, C], F16, kind="ExternalInput")
    ones = nc.dram_tensor("ones", [1, 128], F16, kind="ExternalInput")
    offs = nc.dram_tensor("offs", [128, MAX_REP, N_TT], I32,
                          kind="ExternalInput")
    sflags = nc.dram_tensor("sflags", [1, 4], I32, kind="ExternalInput")
    if ln_affine:
        lng = nc.dram_tensor("lng", [1, C], F32, kind="ExternalInput")
        lnb = nc.dram_tensor("lnb", [1, C], F32, kind="ExternalInput")
    if ln_bias_pad:
        pad_offs = nc.dram_tensor("pad_offs", [128, N_TT], I32,
                                  kind="ExternalInput")
    out_t = nc.dram_tensor("out", [T_SRC, C], F32, kind="ExternalOutput")
    out_full = out_t.ap()

    dsem = nc.alloc_semaphore("dsem")

    with tile.TileContext(nc) as tc, ExitStack() as ctx:
        const = ctx.enter_context(tc.tile_pool(name="const", bufs=1))
        zpool = ctx.enter_context(tc.tile_pool(name="z", bufs=N_TT))
        psum = ctx.enter_context(tc.tile_pool(name="psum", bufs=6,
                                              space="PSUM"))
        stats = ctx.enter_context(tc.tile_pool(name="stats", bufs=4))

        # ---- resident inputs. Order: the first matmul's operands, then
        # the small constants (needed from the 9th matmul / first scatter
        # on — early enough, but off the head of the DMA queue), then the
        # bulk, so the PE pipeline starts as soon as possible. ----
        tgtT_sb = const.tile([128, 8, T_TGT], F16)
        tgtT_r = tgtT.ap().rearrange("(m p) t -> p m t", p=128)
        wT_sb = const.tile([128, 8, C], F16)
        wT_r = wT.ap().rearrange("(m p) o -> p m o", p=128)
        QT = T_TGT // 8
        nc.sync.dma_start(wT_sb[:, 0, :], wT_r[:, 0, :])
        nc.sync.dma_start(tgtT_sb[:, :, 0:128], tgtT_r[:, :, 0:128])
        cb_sb = const.tile([1, C], F16)
        nc.sync.dma_start(cb_sb[:], cb.ap())
        ones_sb = const.tile([1, 128], F16)
        nc.sync.dma_start(ones_sb[:], ones.ap())
        offs_sb = const.tile([128, MAX_REP, N_TT], I32)
        nc.sync.dma_start(offs_sb[:], offs.ap())
        sflags_sb = const.tile([1, 4], I32)
        nc.sync.dma_start(sflags_sb[:], sflags.ap())
        eps_sb = const.tile([128, 1], F32)
        nc.vector.memset(eps_sb[:], LN_EPS)
        for m in range(1, 8):
            nc.sync.dma_start(wT_sb[:, m, :], wT_r[:, m, :])
        nc.sync.dma_start(tgtT_sb[:, :, 128:QT], tgtT_r[:, :, 128:QT])
        for q in range(1, 8):
            nc.sync.dma_start(tgtT_sb[:, :, q * QT:(q + 1) * QT],
                              tgtT_r[:, :, q * QT:(q + 1) * QT])

        pass

        if ln_affine:
            g_sb = const.tile([128, C], F32)
            nc.sync.dma_start(g_sb[:], lng.ap().to_broadcast([128, C]))
            b_sb = const.tile([128, C], F32)
            nc.sync.dma_start(b_sb[:], lnb.ap().to_broadcast([128, C]))
        if ln_bias_pad:
            pad_offs_sb = const.tile([128, N_TT], I32)
            nc.sync.dma_start(pad_offs_sb[:], pad_offs.ap())

        # ---- per token-tile compute ----
        z_tiles = []
        for ti in range(N_TT):
            t_lo = ti * 128
            ps = [psum.tile([128, 512], F32, tag="ps", name=f"ps{ti}_{o}")
                  for o in range(2)]
            for o in range(2):
                for m in range(8):
                    nc.tensor.matmul(
                        ps[o][:],
                        lhsT=tgtT_sb[:, m, t_lo:t_lo + 128],
                        rhs=wT_sb[:, m, o * 512:(o + 1) * 512],
                        start=(m == 0), stop=False)
                # bias via K=1 matmul: ones[1,128].T @ conv_b[1,512]
                nc.tensor.matmul(
                    ps[o][:],
                    lhsT=ones_sb[:, :],
                    rhs=cb_sb[:, o * 512:(o + 1) * 512],
                    start=False, stop=True)

            st = stats.tile([128, 2, 6], F32, tag="st")
            nc.vector.bn_stats(st[:, 0, :], ps[0][:])
            nc.vector.bn_stats(st[:, 1, :], ps[1][:])
            mv = stats.tile([128, 2], F32, tag="mv")
            nc.vector.bn_aggr(mv[:], st[:])
            kt = stats.tile([128, 1], F32, tag="kt")
            nc.scalar.activation(kt[:], mv[:, 1:2],
                                 mybir.ActivationFunctionType.Sqrt,
                                 bias=eps_sb[:], scale=1.0)
            nc.vector.reciprocal(kt[:], kt[:])
            nmk = stats.tile([128, 1], F32, tag="nmk")
            nc.vector.tensor_tensor(nmk[:], mv[:, 0:1], kt[:],
                                    op=mybir.AluOpType.mult)
            nc.scalar.mul(nmk[:], nmk[:], -1.0)

            z = zpool.tile([128, C], F32, tag="z", name=f"z{ti}")
            nc.scalar.activation(z[:, 0:512], ps[0][:],
                                 mybir.ActivationFunctionType.Identity,
                                 bias=nmk[:], scale=kt[:])
            nc.vector.tensor_scalar(out=z[:, 512:1024], in0=ps[1][:],
                                    scalar1=mv[:, 0:1], scalar2=kt[:],
                                    op0=mybir.AluOpType.subtract,
                                    op1=mybir.AluOpType.mult)
            if ln_affine:
                nc.vector.tensor_tensor(z[:], z[:], g_sb[:],
                                        op=mybir.AluOpType.mult)
                nc.vector.tensor_tensor(z[:], z[:], b_sb[:],
                                        op=mybir.AluOpType.add)
            z_tiles.append(z)

        # ---- replicated scatter stream ----
        # copy c of z row t -> out row start[t] + c; OOB entries skipped.
        # Grouped into criticals of GRP token tiles: each group's Tile-level
        # dependency is only its own z tiles, so scatters overlap the rest
        # of the compute; inside a critical the scheduler adds no per-call
        # completion waits (the 2.6x serialization fix).
        GRPS = [(g, 2) for g in range(0, N_TT, 2)]

        def scat(ti, c):
            nc.gpsimd.indirect_dma_start(
                out=out_full,
                out_offset=bass.IndirectOffsetOnAxis(
                    ap=offs_sb[:, c, ti:ti + 1], axis=0),
                in_=z_tiles[ti][:],
                in_offset=None,
                bounds_check=T_SRC - 1,
                oob_is_err=False).then_inc(dsem, 16)

        # registers: active group count, max repeat, dsem completion target
        with nc.gpsimd.register("rg") as rg, \
             nc.gpsimd.register("rc") as rc, \
             nc.gpsimd.register("rt") as rt:
            for g, (g0, gsz) in enumerate(GRPS):
                with tc.tile_critical(no_gpsimd_drain=True):
                    if g == 0:
                        nc.gpsimd.reg_load(rg, sflags_sb[0:1, 0:1])
                        nc.gpsimd.reg_load(rc, sflags_sb[0:1, 1:2])
                        nc.gpsimd.reg_load(rt, sflags_sb[0:1, 2:3])
                    with nc.gpsimd.If_cmp(rg, g, "IS_GT"):
                        for ti in range(g0, g0 + gsz):
                            scat(ti, 0)
                            scat(ti, 1)
                        with nc.gpsimd.If_cmp(rc, 2, "IS_GT"):
                            for ti in range(g0, g0 + gsz):
                                scat(ti, 2)
                            with nc.gpsimd.If_cmp(rc, 3, "IS_GT"):
                                for ti in range(g0, g0 + gsz):
                                    scat(ti, 3)
                    if g == len(GRPS) - 1:
                        # close open If bodies, then the unconditional
                        # pad scatters + completion wait (dsem proves all
                        # scatter DMAs landed; no extra critical needed)
                        nc.gpsimd.end_ifs()
                        if ln_bias_pad:
                            for e in range(N_TT):
                                nc.gpsimd.indirect_dma_start(
                                    out=out_full,
                                    out_offset=bass.IndirectOffsetOnAxis(
                                        ap=pad_offs_sb[:, e:e + 1], axis=0),
                                    in_=b_sb[:],
                                    in_offset=None,
                                    bounds_check=T_SRC - 1,
                                    oob_is_err=False).then_inc(dsem, 16)
                        nc.gpsimd.wait_ge(dsem, rt)

    nc.compile()
    return nc


_PLANS: dict = {}


def _get_plan(ln_affine: bool, ln_bias_pad: bool):
    key = (ln_affine, ln_bias_pad)
    if key not in _PLANS:
        _PLANS[key] = _build_nc(ln_affine, ln_bias_pad)
    return _PLANS[key]


def _host_prep(src_mask, tgt_mask):
    """Per-row scatter offset tables from the ragged masks."""
    src_sizes = src_mask.astype(np.int64).sum(-1)
    tgt_sizes = tgt_mask.astype(np.int64).sum(-1)
    offs = np.full((B, 128, MAX_REP, N_TT), OOB_SENTINEL, dtype=np.int32)
    pad_offs = np.full((B, 128, N_TT), OOB_SENTINEL, dtype=np.int32)
    flags = np.zeros((B, 3), dtype=np.int64)  # [n_active, max_rep, tiles]
    j = np.arange(T_SRC, dtype=np.int64)
    for b in range(B):
        ss, ts = int(src_sizes[b]), int(tgt_sizes[b])
        idx = (j * ts) // max(ss, 1)
        idx = np.clip(idx, 0, max(ts, 1) - 1)
        idxv = idx[:ss]
        counts = np.bincount(idxv, minlength=T_TGT)  # repeats per z row
        starts = np.concatenate(([0], np.cumsum(counts)[:-1]))
        tab = np.full((T_TGT, MAX_REP), OOB_SENTINEL, dtype=np.int64)
        for c in range(MAX_REP):
            m = counts > c
            tab[m, c] = starts[m] + c
        # t = 128*k + p  ->  [p, c, k]
        offs[b] = tab.reshape(N_TT, 128, MAX_REP).transpose(1, 2, 0)
        t_used = int(np.nonzero(counts)[0].max()) + 1 if counts.any() else 0
        tiles_used = -(-t_used // 128)
        grp_starts = list(range(0, N_TT, 2))
        grp_sizes = [2] * (N_TT // 2)
        n_active = sum(1 for s0 in grp_starts if s0 < tiles_used)
        flags[b, 0] = n_active                  # active scatter groups
        flags[b, 1] = int(counts.max())         # max repeat count
        flags[b, 2] = sum(sz for s0, sz in zip(grp_starts, grp_sizes)
                          if s0 < tiles_used)   # tiles emitted
        npad = T_SRC - ss
        padtab = np.full(128 * N_TT, OOB_SENTINEL, dtype=np.int64)
        padtab[:npad] = ss + np.arange(npad)
        pad_offs[b] = padtab.reshape(N_TT, 128).transpose(1, 0)
    return offs, pad_offs, flags


def _prepare(src, src_mask, tgt, tgt_mask, conv_w, conv_b, ln_g, ln_b):
    """Host-side sharding/layout prep. Returns (nc, in_maps, src_mask)."""
    src_mask = np.asarray(src_mask).astype(np.int32)
    tgt_mask = np.asarray(tgt_mask).astype(np.int32)
    tgt = np.asarray(tgt, dtype=np.float32)
    conv_w = np.asarray(conv_w, dtype=np.float32)
    conv_b = np.asarray(conv_b, dtype=np.float32)
    ln_g = np.asarray(ln_g, dtype=np.float32)
    ln_b = np.asarray(ln_b, dtype=np.float32)

    ln_affine = not (np.all(ln_g == 1.0) and np.all(ln_b == 0.0))
    ln_bias_pad = not np.all(ln_b == 0.0)
    nc = _get_plan(ln_affine, ln_bias_pad)

    offs, pad_offs, flags = _host_prep(src_mask, tgt_mask)
    wT = np.ascontiguousarray(conv_w.T).astype(MM_NP_DT)  # [C_in, C_out]
    cb = conv_b.reshape(1, C).astype(MM_NP_DT)
    ones = np.ones((1, 128), dtype=MM_NP_DT)

    sflags = np.zeros((B, 1, 4), dtype=np.int32)
    for b in range(B):
        ng, mr, nt = int(flags[b, 0]), int(flags[b, 1]), int(flags[b, 2])
        per_tile = 2 + (1 if mr >= 3 else 0) + (1 if mr >= 4 else 0)
        n_calls = nt * per_tile + (N_TT if ln_bias_pad else 0)
        sflags[b] = np.array([[ng, mr, 16 * n_calls, 0]], dtype=np.int32)

    in_maps = []
    for b in range(B):
        m = {
            "tgtT": np.ascontiguousarray(tgt[b].T).astype(MM_NP_DT),
            "wT": wT,
            "cb": cb,
            "offs": offs[b],
            "ones": ones,
            "sflags": sflags[b],
        }
        if ln_affine:
            m["lng"] = ln_g.reshape(1, C)
            m["lnb"] = ln_b.reshape(1, C)
        if ln_bias_pad:
            m["pad_offs"] = pad_offs[b]
        in_maps.append(m)
    return nc, in_maps, src_mask


def kernel(src, src_mask, tgt, tgt_mask, conv_w, conv_b, ln_g, ln_b):
    nc, in_maps, src_mask = _prepare(src, src_mask, tgt, tgt_mask,
                                     conv_w, conv_b, ln_g, ln_b)
    res = run_bass_kernel_spmd(nc, in_maps, list(range(N_CORES)))
    out = np.stack([res.results[i]["out"] for i in range(N_CORES)])
    return (out, src_mask)


# revision 27
# speedup vs baseline: 1.1687x; 1.0024x over previous
"""Trainium2 Bass kernel for nn_AlignModalities (ragged nearest-neighbor
resize of conv1x1(tgt) + LayerNorm).

Sharding: pure data parallel — batch row b -> core b (B == 8 == n_cores).

Per-core device program (SPMD — identical program, per-core data):
  1. y[t, o] = sum_c tgt[t, c] * W[o, c] + b[o]   (fp16-operand matmuls on
     PE accumulating in fp32 PSUM, bias folded in as a K=1 matmul against a
     ones-row)
  2. LayerNorm over channels via bn_stats/bn_aggr + per-partition
     scale/bias on the scalar engine (z = (y - mu) * rstd)
  3. Ragged nearest-neighbor row replication straight out of SBUF with
     indirect scatter-DMA: copy c of z-row t lands at out row start[t] + c;
     entries with copy index >= repeat count carry an OOB sentinel and are
     silently skipped (bounds_check).  Out rows past src_size are never
     written (output buffers are pre-zeroed).  All scatters live in one
     tile_critical with manual semaphores: paced by the per-tile z
     semaphore, completion via a dedicated DMA semaphore — this avoids the
     per-call write-after-write completion stalls the scheduler would
     otherwise insert on the shared output tensor (3.6us -> 1.6us/call).

All ragged/data-dependent structure lives in input tensors (offset tables),
so one compiled NEFF serves all 8 cores.
"""

from contextlib import ExitStack

import numpy as np

import concourse.bacc as bacc
import concourse.bass as bass
import concourse.tile as tile
from concourse import mybir
from concourse.bass_utils import run_bass_kernel_spmd

# Problem shape (hardcoded per contract).
B = 8
T_SRC = 4096
T_TGT = 2048
C = 1024
LN_EPS = 1e-5
N_CORES = 8

MAX_REP = 4  # max nearest-neighbor repeat count: ceil(T_SRC / (T_TGT/2))
N_TT = T_TGT // 128  # 16 token tiles of 128 rows
OOB_SENTINEL = 1 << 20

F32 = mybir.dt.float32
F16 = mybir.dt.float16
I32 = mybir.dt.int32

MM_NP_DT = np.float16  # host-side dtype for matmul operands


def _build_nc(ln_affine: bool, ln_bias_pad: bool):
    """Emit the single-core program. ln_affine: apply ln_g/ln_b elementwise.
    ln_bias_pad: write ln_b into out rows >= src_size (only needed when
    ln_b != 0)."""
    nc = bacc.Bacc("TRN2", target_bir_lowering=False, debug=False,
                   num_devices=N_CORES)

    tgtT = nc.dram_tensor("tgtT", [C, T_TGT], F16, kind="ExternalInput")
    wT = nc.dram_tensor("wT", [C, C], F16, kind="ExternalInput")
    cb = nc.dram_tensor("cb", [1, C], F16, kind="ExternalInput")
    ones = nc.dram_tensor("ones", [1, 128], F16, kind="ExternalInput")
    offs = nc.dram_tensor("offs", [128, MAX_REP, N_TT], I32,
                          kind="ExternalInput")
    sflags = nc.dram_tensor("sflags", [1, 4], I32, kind="ExternalInput")
    if ln_affine:
        lng = nc.dram_tensor("lng", [1, C], F32, kind="ExternalInput")
        lnb = nc.dram_tensor("lnb", [1, C], F32, kind="ExternalInput")
    if ln_bias_pad:
        pad_offs = nc.dram_tensor("pad_offs", [128, N_TT], I32,
                                  kind="ExternalInput")
    out_t = nc.dram_tensor("out", [T_SRC, C], F32, kind="ExternalOutput")
    out_full = out_t.ap()

    dsem = nc.alloc_semaphore("dsem")

    with tile.TileContext(nc) as tc, ExitStack() as ctx:
        const = ctx.enter_context(tc.tile_pool(name="const", bufs=1))
        zpool = ctx.enter_context(tc.tile_pool(name="z", bufs=N_TT))
        psum = ctx.enter_context(tc.tile_pool(name="psum", bufs=6,
                                              space="PSUM"))
        stats = ctx.enter_context(tc.tile_pool(name="stats", bufs=4))

        # ---- resident inputs. Order: the first matmul's operands, then
        # the small constants (needed from the 9th matmul / first scatter
        # on — early enough, but off the head of the DMA queue), then the
        # bulk, so the PE pipeline starts as soon as possible. ----
        tgtT_sb = const.tile([128, 8, T_TGT], F16)
        tgtT_r = tgtT.ap().rearrange("(m p) t -> p m t", p=128)
        wT_sb = const.tile([128, 8, C], F16)
        wT_r = wT.ap().rearrange("(m p) o -> p m o", p=128)
        QT = T_TGT // 8
        nc.sync.dma_start(wT_sb[:, 0, :], wT_r[:, 0, :])
        nc.sync.dma_start(tgtT_sb[:, :, 0:128], tgtT_r[:, :, 0:128])
        cb_sb = const.tile([1, C], F16)
        nc.sync.dma_start(cb_sb[:], cb.ap())
        ones_sb = const.tile([1, 128], F16)
        nc.sync.dma_start(ones_sb[:], ones.ap())
        offs_sb = const.tile([128, MAX_REP, N_TT], I32)
        nc.sync.dma_start(offs_sb[:], offs.ap())
        sflags_sb = const.tile([1, 4], I32)
        nc.sync.dma_start(sflags_sb[:], sflags.ap())
        eps_sb = const.tile([128, 1], F32)
        nc.vector.memset(eps_sb[:], LN_EPS)
        for m in range(1, 8):
            nc.sync.dma_start(wT_sb[:, m, :], wT_r[:, m, :])
        nc.sync.dma_start(tgtT_sb[:, :, 128:QT], tgtT_r[:, :, 128:QT])
        for q in range(1, 8):
            nc.sync.dma_start(tgtT_sb[:, :, q * QT:(q + 1) * QT],
                              tgtT_r[:, :, q * QT:(q + 1) * QT])

        pass

        if ln_affine:
            g_sb = const.tile([128, C], F32)
            nc.sync.dma_start(g_sb[:], lng.ap().to_broadcast([128, C]))
            b_sb = const.tile([128, C], F32)
            nc.sync.dma_start(b_sb[:], lnb.ap().to_broadcast([128, C]))
        if ln_bias_pad:
            pad_offs_sb = const.tile([128, N_TT], I32)
            nc.sync.dma_start(pad_offs_sb[:], pad_offs.ap())

        # ---- per token-tile compute ----
        z_tiles = []
        for ti in range(N_TT):
            t_lo = ti * 128
            ps = [psum.tile([128, 512], F32, tag="ps", name=f"ps{ti}_{o}")
                  for o in range(2)]
            for o in range(2):
                for m in range(8):
                    nc.tensor.matmul(
                        ps[o][:],
                        lhsT=tgtT_sb[:, m, t_lo:t_lo + 128],
                        rhs=wT_sb[:, m, o * 512:(o + 1) * 512],
                        start=(m == 0), stop=False)
                # bias via K=1 matmul: ones[1,128].T @ conv_b[1,512]
                nc.tensor.matmul(
                    ps[o][:],
                    lhsT=ones_sb[:, :],
                    rhs=cb_sb[:, o * 512:(o + 1) * 512],
                    start=False, stop=True)

            st = stats.tile([128, 2, 6], F32, tag="st")
            nc.vector.bn_stats(st[:, 0, :], ps[0][:])
            nc.vector.bn_stats(st[:, 1, :], ps[1][:])
            mv = stats.tile([128, 2], F32, tag="mv")
            nc.vector.bn_aggr(mv[:], st[:])
            kt = stats.tile([128, 1], F32, tag="kt")
            nc.scalar.activation(kt[:], mv[:, 1:2],
                                 mybir.ActivationFunctionType.Sqrt,
                                 bias=eps_sb[:], scale=1.0)
            nc.vector.reciprocal(kt[:], kt[:])
            nmk = stats.tile([128, 1], F32, tag="nmk")
            nc.vector.tensor_tensor(nmk[:], mv[:, 0:1], kt[:],
                                    op=mybir.AluOpType.mult)
            nc.scalar.mul(nmk[:], nmk[:], -1.0)

            z = zpool.tile([128, C], F32, tag="z", name=f"z{ti}")
            nc.scalar.activation(z[:, 0:512], ps[0][:],
                                 mybir.ActivationFunctionType.Identity,
                                 bias=nmk[:], scale=kt[:])
            nc.vector.tensor_scalar(out=z[:, 512:1024], in0=ps[1][:],
                                    scalar1=mv[:, 0:1], scalar2=kt[:],
                                    op0=mybir.AluOpType.subtract,
                                    op1=mybir.AluOpType.mult)
            if ln_affine:
                nc.vector.tensor_tensor(z[:], z[:], g_sb[:],
                                        op=mybir.AluOpType.mult)
                nc.vector.tensor_tensor(z[:], z[:], b_sb[:],
                                        op=mybir.AluOpType.add)
            z_tiles.append(z)

        # ---- replicated scatter stream ----
        # copy c of z row t -> out row start[t] + c; OOB entries skipped.
        # Grouped into criticals of GRP token tiles: each group's Tile-level
        # dependency is only its own z tiles, so scatters overlap the rest
        # of the compute; inside a critical the scheduler adds no per-call
        # completion waits (the 2.6x serialization fix).
        # pairs, except the last two tiles go solo: their scatters fire
        # per-tile instead of bunching all four calls behind the final z
        GRPS = [(g, 2) for g in range(0, N_TT - 2, 2)] + [(N_TT - 2, 1),
                                                          (N_TT - 1, 1)]

        def scat(ti, c):
            nc.gpsimd.indirect_dma_start(
                out=out_full,
                out_offset=bass.IndirectOffsetOnAxis(
                    ap=offs_sb[:, c, ti:ti + 1], axis=0),
                in_=z_tiles[ti][:],
                in_offset=None,
                bounds_check=T_SRC - 1,
                oob_is_err=False).then_inc(dsem, 16)

        # registers: active group count, max repeat, dsem completion target
        with nc.gpsimd.register("rg") as rg, \
             nc.gpsimd.register("rc") as rc, \
             nc.gpsimd.register("rt") as rt:
            for g, (g0, gsz) in enumerate(GRPS):
                with tc.tile_critical(no_gpsimd_drain=True):
                    if g == 0:
                        nc.gpsimd.reg_load(rg, sflags_sb[0:1, 0:1])
                        nc.gpsimd.reg_load(rc, sflags_sb[0:1, 1:2])
                        nc.gpsimd.reg_load(rt, sflags_sb[0:1, 2:3])
                    with nc.gpsimd.If_cmp(rg, g, "IS_GT"):
                        for ti in range(g0, g0 + gsz):
                            scat(ti, 0)
                            scat(ti, 1)
                        with nc.gpsimd.If_cmp(rc, 2, "IS_GT"):
                            for ti in range(g0, g0 + gsz):
                                scat(ti, 2)
                            with nc.gpsimd.If_cmp(rc, 3, "IS_GT"):
                                for ti in range(g0, g0 + gsz):
                                    scat(ti, 3)
                    if g == len(GRPS) - 1:
                        # close open If bodies, then the unconditional
                        # pad scatters + completion wait (dsem proves all
                        # scatter DMAs landed; no extra critical needed)
                        nc.gpsimd.end_ifs()
                        if ln_bias_pad:
                            for e in range(N_TT):
                                nc.gpsimd.indirect_dma_start(
                                    out=out_full,
                                    out_offset=bass.IndirectOffsetOnAxis(
                                        ap=pad_offs_sb[:, e:e + 1], axis=0),
                                    in_=b_sb[:],
                                    in_offset=None,
                                    bounds_check=T_SRC - 1,
                                    oob_is_err=False).then_inc(dsem, 16)
                        nc.gpsimd.wait_ge(dsem, rt)

    nc.compile()
    return nc


_PLANS: dict = {}


def _get_plan(ln_affine: bool, ln_bias_pad: bool):
    key = (ln_affine, ln_bias_pad)
    if key not in _PLANS:
        _PLANS[key] = _build_nc(ln_affine, ln_bias_pad)
    return _PLANS[key]


def _host_prep(src_mask, tgt_mask):
    """Per-row scatter offset tables from the ragged masks."""
    src_sizes = src_mask.astype(np.int64).sum(-1)
    tgt_sizes = tgt_mask.astype(np.int64).sum(-1)
    offs = np.full((B, 128, MAX_REP, N_TT), OOB_SENTINEL, dtype=np.int32)
    pad_offs = np.full((B, 128, N_TT), OOB_SENTINEL, dtype=np.int32)
    flags = np.zeros((B, 3), dtype=np.int64)  # [n_active, max_rep, tiles]
    j = np.arange(T_SRC, dtype=np.int64)
    for b in range(B):
        ss, ts = int(src_sizes[b]), int(tgt_sizes[b])
        idx = (j * ts) // max(ss, 1)
        idx = np.clip(idx, 0, max(ts, 1) - 1)
        idxv = idx[:ss]
        counts = np.bincount(idxv, minlength=T_TGT)  # repeats per z row
        starts = np.concatenate(([0], np.cumsum(counts)[:-1]))
        tab = np.full((T_TGT, MAX_REP), OOB_SENTINEL, dtype=np.int64)
        for c in range(MAX_REP):
            m = counts > c
            tab[m, c] = starts[m] + c
        # t = 128*k + p  ->  [p, c, k]
        offs[b] = tab.reshape(N_TT, 128, MAX_REP).transpose(1, 2, 0)
        t_used = int(np.nonzero(counts)[0].max()) + 1 if counts.any() else 0
        tiles_used = -(-t_used // 128)
        grp_starts = list(range(0, N_TT - 2, 2)) + [N_TT - 2, N_TT - 1]
        grp_sizes = [2] * (N_TT // 2 - 1) + [1, 1]
        n_active = sum(1 for s0 in grp_starts if s0 < tiles_used)
        flags[b, 0] = n_active                  # active scatter groups
        flags[b, 1] = int(counts.max())         # max repeat count
        flags[b, 2] = sum(sz for s0, sz in zip(grp_starts, grp_sizes)
                          if s0 < tiles_used)   # tiles emitted
        npad = T_SRC - ss
        padtab = np.full(128 * N_TT, OOB_SENTINEL, dtype=np.int64)
        padtab[:npad] = ss + np.arange(npad)
        pad_offs[b] = padtab.reshape(N_TT, 128).transpose(1, 0)
    return offs, pad_offs, flags


def _prepare(src, src_mask, tgt, tgt_mask, conv_w, conv_b, ln_g, ln_b):
    """Host-side sharding/layout prep. Returns (nc, in_maps, src_mask)."""
    src_mask = np.asarray(src_mask).astype(np.int32)
    tgt_mask = np.asarray(tgt_mask).astype(np.int32)
    tgt = np.asarray(tgt, dtype=np.float32)
    conv_w = np.asarray(conv_w, dtype=np.float32)
    conv_b = np.asarray(conv_b, dtype=np.float32)
    ln_g = np.asarray(ln_g, dtype=np.float32)
    ln_b = np.asarray(ln_b, dtype=np.float32)

    ln_affine = not (np.all(ln_g == 1.0) and np.all(ln_b == 0.0))
    ln_bias_pad = not np.all(ln_b == 0.0)
    nc = _get_plan(ln_affine, ln_bias_pad)

    offs, pad_offs, flags = _host_prep(src_mask, tgt_mask)
    wT = np.ascontiguousarray(conv_w.T).astype(MM_NP_DT)  # [C_in, C_out]
    cb = conv_b.reshape(1, C).astype(MM_NP_DT)
    ones = np.ones((1, 128), dtype=MM_NP_DT)

    sflags = np.zeros((B, 1, 4), dtype=np.int32)
    for b in range(B):
        ng, mr, nt = int(flags[b, 0]), int(flags[b, 1]), int(flags[b, 2])
        per_tile = 2 + (1 if mr >= 3 else 0) + (1 if mr >= 4 else 0)
        n_calls = nt * per_tile + (N_TT if ln_bias_pad else 0)
        sflags[b] = np.array([[ng, mr, 16 * n_calls, 0]], dtype=np.int32)

    in_maps = []
    for b in range(B):
        m = {
            "tgtT": np.ascontiguousarray(tgt[b].T).astype(MM_NP_DT),
            "wT": wT,
            "cb": cb,
            "offs": offs[b],
            "ones": ones,
            "sflags": sflags[b],
        }
        if ln_affine:
            m["lng"] = ln_g.reshape(1, C)
            m["lnb"] = ln_b.reshape(1, C)
        if ln_bias_pad:
            m["pad_offs"] = pad_offs[b]
        in_maps.append(m)
    return nc, in_maps, src_mask


def kernel(src, src_mask, tgt, tgt_mask, conv_w, conv_b, ln_g, ln_b):
    nc, in_maps, src_mask = _prepare(src, src_mask, tgt, tgt_mask,
                                     conv_w, conv_b, ln_g, ln_b)
    res = run_bass_kernel_spmd(nc, in_maps, list(range(N_CORES)))
    out = np.stack([res.results[i]["out"] for i in range(N_CORES)])
    return (out, src_mask)
